# revision 6
# baseline (speedup 1.0000x reference)
"""GNN message-passing (NNConv x4 + MLP readout) on 8 Trainium2 cores.

Sharding: edges by DESTINATION node range (core c owns nodes
[c*OWN,(c+1)*OWN)) so segment sums are core-local; x is AllGathered each
layer. Per-edge weight MLP on the PE array; the per-edge bilinear einsum on
DVE via broadcast-AP tensor_tensor; segment-sum via dma_scatter_add split
into occurrence layers (unique indices per call; colliding CCE R-M-W loses
updates otherwise), serialized with explicit deps.
"""

import sys
import types
import numpy as np


def _install_axon_hooks():
    if "antenv.axon_hooks" in sys.modules:
        return
    hooks = types.ModuleType("antenv.axon_hooks")
    _h = [None]
    hooks.set_axon_ntff_profile_hook = lambda h: _h.__setitem__(0, h)
    hooks.get_axon_ntff_profile_hook = lambda: _h[0]
    sys.modules["antenv.axon_hooks"] = hooks
    try:
        import antenv
        antenv.axon_hooks = hooks
        from trn_agent_boot.trn_boot import _ntff_profile_via_ctypes
        hooks.set_axon_ntff_profile_hook(
            _ntff_profile_via_ctypes("/opt/axon/libaxon_pjrt.so"))
    except Exception:
        pass


_install_axon_hooks()

import concourse.bass_utils as _bu  # noqa: E402
_bu.upload_artifacts = lambda tmpdir: "local://skipped"

import concourse.bass as bass  # noqa: E402
import concourse.mybir as mybir  # noqa: E402
import concourse.tile as tile  # noqa: E402
from concourse import bacc  # noqa: E402
from concourse.bass_utils import run_bass_kernel_spmd  # noqa: E402
from concourse.tile_rust import add_dep_helper  # noqa: E402
from concourse.masks import make_identity  # noqa: E402

F32 = mybir.dt.float32
I32 = mybir.dt.int32
I16 = mybir.dt.int16
AF = mybir.ActivationFunctionType
ALU = mybir.AluOpType

NCORES = 8
D = 32
HID = D * D
TB = 8  # einsum tile batch


def _ceil_to(x, m):
    return (x + m - 1) // m * m


# ---------------------------------------------------------------------------
def _host_prep(var_node_features, con_node_features, node_types, edge_index,
               edge_types, assoc_var, assoc_con, params):
    NV = var_node_features.shape[0]
    N = node_types.shape[0]
    assert N % NCORES == 0 and NV % NCORES == 0
    OWN = N // NCORES
    OWN_PAD = _ceil_to(OWN, 512)
    NWIN = OWN_PAD // 128
    ACC_ROWS = _ceil_to(OWN_PAD + 1, 128)
    DUMMY = OWN_PAD
    VARS = NV // NCORES
    VAR_PAD = _ceil_to(VARS, 512)

    src = np.asarray(edge_index[0], np.int64)
    dst = np.asarray(edge_index[1], np.int64)
    ea = np.asarray(edge_types, np.float32)

    core_layers = []
    for c in range(NCORES):
        lo, hi = c * OWN, (c + 1) * OWN
        eids = np.nonzero((dst >= lo) & (dst < hi))[0]
        eids = eids[np.argsort(dst[eids], kind="stable")]
        dl = dst[eids] - lo
        n = len(eids)
        occ = np.zeros(n, np.int64)
        for i in range(1, n):
            occ[i] = occ[i - 1] + 1 if dl[i] == dl[i - 1] else 0
        nlay = int(occ.max()) + 1 if n else 0
        core_layers.append([eids[occ == k] for k in range(nlay)])

    NLAY = max(len(ls) for ls in core_layers)
    chunks_k = []
    for k in range(NLAY):
        mx = max((len(ls[k]) if k < len(ls) else 0) for ls in core_layers)
        chunks_k.append(max(1, (mx + 127) // 128))
    tot = sum(chunks_k) * 128
    chunks_k[-1] += (_ceil_to(tot, 2048) - tot) // 128
    E_PAD = sum(chunks_k) * 128
    NCHUNK = E_PAD // 128
    E4 = E_PAD // 4
    assert E4 % 512 == 0

    layer_ranges = []
    clo = 0
    for k in range(NLAY):
        layer_ranges.append((clo, chunks_k[k]))
        clo += chunks_k[k]

    vnf = np.asarray(var_node_features, np.float32)
    cnf = np.asarray(con_node_features, np.float32)

    per_core = []
    for c in range(NCORES):
        lo, hi = c * OWN, (c + 1) * OWN
        slot_src = np.zeros(E_PAD, np.int64)
        slot_dstl = np.full(E_PAD, DUMMY, np.int64)
        slot_ea = np.zeros((E_PAD, 2), np.float32)
        layers = core_layers[c]
        for k in range(min(NLAY, len(layers))):
            base = layer_ranges[k][0] * 128
            eids = layers[k]
            slot_src[base:base + len(eids)] = src[eids]
            slot_dstl[base:base + len(eids)] = dst[eids] - lo
            slot_ea[base:base + len(eids)] = ea[eids]
        srcidx = slot_src.reshape(NCHUNK, 128).T.astype(np.int32).copy()
        sc16 = slot_dstl.astype(np.int16).reshape(E_PAD // 16, 16).T.copy()
        sc16 = np.ascontiguousarray(np.tile(sc16, (8, 1)))
        eaT = np.ascontiguousarray(slot_ea.T)

        nt_own = np.arange(lo, hi)
        is_var = nt_own < NV
        feat = np.zeros((OWN_PAD, 2), np.float32)
        feat[:OWN][is_var] = vnf[nt_own[is_var]]
        feat[:OWN][~is_var] = cnf[nt_own[~is_var] - NV]
        maskv = np.zeros(OWN_PAD, np.float32)
        maskc = np.zeros(OWN_PAD, np.float32)
        maskv[:OWN][is_var] = 1.0
        maskc[:OWN][~is_var] = 1.0
        cnt = np.bincount(dst[(dst >= lo) & (dst < hi)] - lo,
                          minlength=OWN_PAD).astype(np.float32)
        invc = np.zeros(OWN_PAD, np.float32)
        invc[:OWN] = 1.0 / np.maximum(cnt[:OWN], 1.0)

        def _wins(v):
            return np.ascontiguousarray(v.reshape(NWIN, 128).T)

        per_core.append({
            "eaT": eaT, "srcidx": srcidx, "scidx": sc16,
            "featT": np.ascontiguousarray(feat.T),
            "maskv": _wins(maskv), "maskc": _wins(maskc), "invc": _wins(invc),
        })

    def g(*ks):
        x = params
        for k in ks:
            x = x[k]
        return np.ascontiguousarray(np.asarray(x, np.float32))

    P = {}
    for pre, name in (("v", "var_mlp"), ("c", "con_mlp")):
        P[f"{pre}_w1"] = g(name, "W1")
        P[f"{pre}_b1"] = g(name, "b1").reshape(D, 1)
        P[f"{pre}_w2"] = g(name, "W2")
        P[f"{pre}_b2"] = g(name, "b2").reshape(1, D)
    for li, name in enumerate(("conv1", "conv2", "conv3", "conv4")):
        P[f"w1_{li}"] = g(name, "nn", "W1")
        P[f"b1r_{li}"] = np.ascontiguousarray(
            np.tile(g(name, "nn", "b1"), 4).reshape(128, 1))
        P[f"w2rep_{li}"] = np.ascontiguousarray(np.tile(g(name, "nn", "W2"), (4, 1)))
        P[f"b2_{li}"] = g(name, "nn", "b2").reshape(1, HID)
        P[f"root_{li}"] = np.ascontiguousarray(np.tile(g(name, "root"), (4, 1)))
        P[f"bias_{li}"] = g(name, "bias").reshape(1, D)
    fc1 = g("fc1", "W")
    P["fc1a"] = np.ascontiguousarray(fc1[:128])
    P["fc1b"] = np.ascontiguousarray(fc1[128:])
    P["fc1bias"] = g("fc1", "b").reshape(1, D)
    P["fc2w"] = g("fc2", "W")
    P["fc2bias"] = g("fc2", "b").reshape(1, D)
    P["fc3w"] = g("fc3", "W")
    P["fc3bias"] = g("fc3", "b").reshape(1, D)
    P["fc6w"] = g("fc6", "W")
    P["fc6bias"] = g("fc6", "b").reshape(1, 2)

    struct = dict(
        N=N, NV=NV, OWN=OWN, OWN_PAD=OWN_PAD, NWIN=NWIN, ACC_ROWS=ACC_ROWS,
        VARS=VARS, VAR_PAD=VAR_PAD, E_PAD=E_PAD, NCHUNK=NCHUNK, E4=E4,
        layer_ranges=layer_ranges,
        b2_zero=[float(np.abs(P[f"b2_{li}"]).max()) == 0.0 for li in range(4)],
        bias_zero=[float(np.abs(P[f"bias_{li}"]).max()) == 0.0 for li in range(4)],
    )
    return per_core, P, struct


# ---------------------------------------------------------------------------
def _build(struct, P):
    S = struct
    OWN, OWN_PAD, NWIN = S["OWN"], S["OWN_PAD"], S["NWIN"]
    ACC_ROWS, VAR_PAD = S["ACC_ROWS"], S["VAR_PAD"]
    E_PAD, NCHUNK, E4 = S["E_PAD"], S["NCHUNK"], S["E4"]
    N = S["N"]
    NB1 = E_PAD // 2048
    NFEAT = OWN_PAD // 512
    NVT = VAR_PAD // 128
    NRC = VAR_PAD // 512
    OP4 = OWN_PAD // 4
    WPC = NWIN // 4

    nc = bacc.Bacc("TRN2", target_bir_lowering=False, debug=False,
                   num_devices=NCORES)

    eaT_d = nc.dram_tensor("eaT", [2, E_PAD], F32, kind="ExternalInput").ap()
    srcidx_d = nc.dram_tensor("srcidx", [128, NCHUNK], I32, kind="ExternalInput").ap()
    scidx_d = nc.dram_tensor("scidx", [128, E_PAD // 16], I16, kind="ExternalInput").ap()
    featT_d = nc.dram_tensor("featT", [2, OWN_PAD], F32, kind="ExternalInput").ap()
    maskv_d = nc.dram_tensor("maskv", [128, NWIN], F32, kind="ExternalInput").ap()
    maskc_d = nc.dram_tensor("maskc", [128, NWIN], F32, kind="ExternalInput").ap()
    invc_d = nc.dram_tensor("invc", [128, NWIN], F32, kind="ExternalInput").ap()
    vrow_d = nc.dram_tensor("vrowidx", [128, NVT], I32, kind="ExternalInput").ap()
    p_d = {k: nc.dram_tensor(k, list(v.shape), F32, kind="ExternalInput").ap()
           for k, v in P.items()}
    out_d = nc.dram_tensor("logits", [VAR_PAD, 2], F32, kind="ExternalOutput").ap()

    xfull = [nc.dram_tensor(f"xfull{l}", [N, D], F32, kind="Internal",
                            addr_space="Shared").ap() for l in range(5)]
    xbounce = [nc.dram_tensor(f"xbounce{l}", [OWN, D], F32, kind="Internal").ap()
               for l in range(5)]
    accum = [nc.dram_tensor(f"accum{l}", [ACC_ROWS, 2 * D], F32,
                            kind="Internal").ap() for l in range(4)]

    rg = [list(range(NCORES))]

    with tile.TileContext(nc) as tc:
        with tc.tile_pool(name="const", bufs=1) as cpool, \
             tc.tile_pool(name="sbuf", bufs=2) as sbuf, \
             tc.tile_pool(name="psA", bufs=2, space="PSUM") as psA, \
             tc.tile_pool(name="psW", bufs=2, space="PSUM") as psW, \
             tc.tile_pool(name="psS", bufs=2, space="PSUM") as psS:

            def pa(shape):
                return psA.tile(shape, F32, tag="pa", space="PSUM", name="pa")

            def pw(shape):
                return psW.tile(shape, F32, tag="pw", space="PSUM", name="pw")

            def pt(shape):
                return psS.tile(shape, F32, tag="pt", space="PSUM", name="pt")

            ident = cpool.tile([128, 128], F32)
            make_identity(nc, ident[:])
            ones_row = cpool.tile([1, 512], F32)
            nc.vector.memset(ones_row[:], 1.0)
            zrows = ACC_ROWS // 128
            zh = (zrows + 3) // 4
            zero_big = cpool.tile([128, zh * 2 * D], F32)
            nc.vector.memset(zero_big[:], 0.0)

            ptile = {}
            for k, v in P.items():
                t = cpool.tile(list(v.shape), F32, tag=f"p_{k}")
                nc.sync.dma_start(out=t[:], in_=p_d[k])
                ptile[k] = t
            maskv_t = cpool.tile([128, NWIN], F32)
            maskc_t = cpool.tile([128, NWIN], F32)
            invc_t = cpool.tile([128, NWIN], F32)
            nc.sync.dma_start(out=maskv_t[:], in_=maskv_d[:])
            nc.sync.dma_start(out=maskc_t[:], in_=maskc_d[:])
            nc.sync.dma_start(out=invc_t[:], in_=invc_d[:])
            srcidx_t = cpool.tile([128, NCHUNK], I32)
            nc.sync.dma_start(out=srcidx_t[:], in_=srcidx_d[:])
            scidx_t = cpool.tile([128, E_PAD // 16], I16)
            nc.sync.dma_start(out=scidx_t[:], in_=scidx_d[:])
            vrow_t = cpool.tile([128, NVT], I32)
            nc.sync.dma_start(out=vrow_t[:], in_=vrow_d[:])

            zero_deps = []
            for l in range(4):
                av = accum[l].tensor.ap().rearrange("(a p) d -> p a d", p=128)
                zs = []
                for q in range(4):
                    qlo = q * zh
                    qhi = min((q + 1) * zh, zrows)
                    if qlo >= qhi:
                        continue
                    zq = nc.sync.dma_start(
                        out=av[:, qlo:qhi, :],
                        in_=zero_big[:, 0:(qhi - qlo) * 2 * D])
                    zs.append(zq)
                zero_deps.append(zs)

            def win_lhsT(slab, w):
                c, wi = w // WPC, w % WPC
                return slab[32 * c:32 * c + 32, wi * 128:(wi + 1) * 128]

            def transpose_to_own(xtile_sb, slab, w):
                tp = pt([D, 128])
                nc.tensor.transpose(out=tp[:], in_=xtile_sb[:], identity=ident[:])
                nc.scalar.activation(out=win_lhsT(slab, w), in_=tp[:], func=AF.Copy)

            with tc.tile_pool(name="big", bufs=1) as big:
                xT_a = big.tile([128, OP4], F32, tag="xT0")
                xT_b = big.tile([128, OP4], F32, tag="xT1")
                xT_own = [xT_a, xT_b]

                # ---------- stage 0 ----------
                x0_written = []
                for ch in range(NFEAT):
                    cs = slice(ch * 512, (ch + 1) * 512)
                    ft = sbuf.tile([2, 512], F32, tag="feat")
                    nc.sync.dma_start(out=ft[:], in_=featT_d[:, cs])
                    res = {}
                    for pre in ("v", "c"):
                        hp = pa([D, 512])
                        nc.tensor.matmul(out=hp[:], lhsT=ptile[f"{pre}_w1"][:],
                                         rhs=ft[:], start=True, stop=True)
                        hs = sbuf.tile([D, 512], F32, tag="s0hs")
                        nc.scalar.activation(out=hs[:], in_=hp[:], func=AF.Relu,
                                             bias=ptile[f"{pre}_b1"][:])
                        op = pa([D, 512])
                        nc.tensor.matmul(out=op[:], lhsT=ptile[f"{pre}_w2"][:],
                                         rhs=hs[:], start=True, stop=False)
                        nc.tensor.matmul(out=op[:], lhsT=ptile[f"{pre}_b2"][:],
                                         rhs=ones_row[:], start=False, stop=True)
                        os_ = sbuf.tile([D, 512], F32, tag="s0os")
                        nc.scalar.activation(out=os_[:], in_=op[:], func=AF.Copy)
                        res[pre] = os_
                    for wi in range(4):
                        w = ch * 4 + wi
                        rows = {}
                        for pre in ("v", "c"):
                            tp = pt([128, D])
                            nc.tensor.transpose(
                                out=tp[:], in_=res[pre][:, wi * 128:(wi + 1) * 128],
                                identity=ident[0:D, 0:D])
                            rt = sbuf.tile([128, D], F32, tag="s0row")
                            nc.scalar.activation(out=rt[:], in_=tp[:], func=AF.Copy)
                            rows[pre] = rt
                        xt = sbuf.tile([128, D], F32, tag="s0x")
                        nc.vector.tensor_scalar_mul(xt[:], rows["v"][:],
                                                    maskv_t[:, w:w + 1])
                        nc.vector.scalar_tensor_tensor(
                            out=xt[:], in0=rows["c"][:], scalar=maskc_t[:, w:w + 1],
                            in1=xt[:], op0=ALU.mult, op1=ALU.add)
                        transpose_to_own(xt, xT_own[0], w)
                        if w * 128 < OWN:
                            hi = min((w + 1) * 128, OWN)
                            wr = nc.sync.dma_start(out=xbounce[0][w * 128:hi, :],
                                                   in_=xt[0:hi - w * 128, :])
                            x0_written.append(wr)

                prev_ag = nc.gpsimd.collective_compute(
                    "AllGather", ALU.bypass, replica_groups=rg,
                    ins=[xbounce[0].opt()], outs=[xfull[0].opt()])
                for wr in x0_written:
                    add_dep_helper(prev_ag.ins, wr.ins, True, "ag after bounce")

                # ---------- conv layers ----------
                for li in range(4):
                    xin = xfull[li]
                    cur_xT, nxt_xT = xT_own[li % 2], xT_own[(li + 1) % 2]

                    rT = big.tile([128, E4], F32, tag="rT")
                    for j in range(NB1):
                        rp = pa([128, 512])
                        et = sbuf.tile([2, 2048], F32, tag="eaT", bufs=1)
                        for c in range(4):
                            nc.sync.dma_start(
                                out=et[:, c * 512:(c + 1) * 512],
                                in_=eaT_d[:, c * E4 + j * 512:c * E4 + (j + 1) * 512])
                        for c in range(4):
                            nc.tensor.matmul(out=rp[32 * c:32 * (c + 1), :],
                                             lhsT=ptile[f"w1_{li}"][:],
                                             rhs=et[:, c * 512:(c + 1) * 512],
                                             start=True, stop=True,
                                             tile_position=(0, 32 * c))
                        nc.scalar.activation(out=rT[:, j * 512:(j + 1) * 512],
                                             in_=rp[:], func=AF.Relu,
                                             bias=ptile[f"b1r_{li}"][:])

                    msg = big.tile([128, NCHUNK * D], F32, tag="msg")
                    msgv = msg[:].rearrange("p (t d) -> p t d", d=D)
                    w2rep = ptile[f"w2rep_{li}"]
                    nbatch = (NCHUNK + TB - 1) // TB
                    for b in range(nbatch):
                        t0, t1 = b * TB, min((b + 1) * TB, NCHUNK)
                        nt = t1 - t0
                        w_sb = big.tile([128, TB * HID], F32, tag="w_sb")
                        xs = sbuf.tile([128, TB * D], F32, tag="xsrc")
                        for t in range(t0, t1):
                            k = t - t0
                            c = (t * 128) // E4
                            col = (t * 128) % E4
                            wp = pw([128, HID])
                            for h in range(2):
                                nc.tensor.matmul(
                                    out=wp[:, h * 512:(h + 1) * 512],
                                    lhsT=rT[32 * c:32 * (c + 1), col:col + 128],
                                    rhs=w2rep[32 * c:32 * (c + 1),
                                              h * 512:(h + 1) * 512],
                                    start=True, stop=True,
                                    tile_position=(32 * c, 0))
                            if not S["b2_zero"][li]:
                                for h in range(2):
                                    nc.tensor.matmul(
                                        out=wp[:, h * 512:(h + 1) * 512],
                                        lhsT=ones_row[:, 0:128],
                                        rhs=ptile[f"b2_{li}"][:, h * 512:(h + 1) * 512],
                                        start=False, stop=True)
                            nc.scalar.activation(out=w_sb[:, k * HID:(k + 1) * HID],
                                                 in_=wp[:], func=AF.Copy)
                            gi = nc.gpsimd.indirect_dma_start(
                                out=xs[:, k * D:(k + 1) * D],
                                out_offset=None,
                                in_=xin[:],
                                in_offset=bass.IndirectOffsetOnAxis(
                                    ap=srcidx_t[:, t:t + 1], axis=0))
                            add_dep_helper(gi.ins, prev_ag.ins, True, "g after ag")
                        wv = w_sb[:].rearrange("p (t h) -> p t h", h=HID)[:, 0:nt, :]
                        xv = xs[:].rearrange("p (t d) -> p t d", d=D)[:, 0:nt, :]
                        mv = msgv[:, t0:t1, :]
                        sa = sbuf.tile([128, TB * D], F32, tag="esA")
                        sb_ = sbuf.tile([128, TB * D], F32, tag="esB")
                        sav = sa[:].rearrange("p (t d) -> p t d", d=D)[:, 0:nt, :]
                        sbv = sb_[:].rearrange("p (t d) -> p t d", d=D)[:, 0:nt, :]
                        cur, nxt = sav, sbv
                        for i in range(D):
                            wsl = wv[:, :, i * D:(i + 1) * D]
                            xb = xv[:, :, i:i + 1].to_broadcast([128, nt, D])
                            if i == 0:
                                nc.vector.tensor_tensor(out=cur, in0=wsl, in1=xb,
                                                        op=ALU.mult)
                            else:
                                dst_ap = mv if i == D - 1 else nxt
                                prod = sbuf.tile([128, TB * D], F32, tag="esP")
                                pv = prod[:].rearrange("p (t d) -> p t d",
                                                       d=D)[:, 0:nt, :]
                                nc.vector.tensor_tensor(out=pv, in0=wsl, in1=xb,
                                                        op=ALU.mult)
                                nc.vector.tensor_tensor(out=dst_ap, in0=pv,
                                                        in1=cur, op=ALU.add)
                                cur, nxt = nxt, cur

                    prev_sc = None
                    for (clo, ccnt) in S["layer_ranges"]:
                        sc = nc.gpsimd.dma_scatter_add(
                            out_ap=accum[li][:, 0:D],
                            in_ap=msgv[:, clo:clo + ccnt, :],
                            idxs_ap=scidx_t[:, clo * 8:(clo + ccnt) * 8],
                            num_idxs=ccnt * 128,
                            num_idxs_reg=ccnt * 128,
                            elem_size=D,
                            elem_step=2 * D)
                        if prev_sc is None:
                            for zq in zero_deps[li]:
                                add_dep_helper(sc.ins, zq.ins, True, "z")
                        else:
                            add_dep_helper(sc.ins, prev_sc.ins, True, "ser sc")
                        prev_sc = sc

                    xw_written = []
                    for w in range(NWIN):
                        at = sbuf.tile([128, D], F32, tag="acc")
                        rd = nc.sync.dma_start(
                            out=at[:], in_=accum[li][w * 128:(w + 1) * 128, 0:D])
                        add_dep_helper(rd.ins, prev_sc.ins, True, "rd after sc")
                        mp = pt([128, D])
                        wc = (w // WPC) * 32
                        nc.tensor.matmul(out=mp[:], lhsT=win_lhsT(cur_xT, w),
                                         rhs=ptile[f"root_{li}"][wc:wc + 32, :],
                                         start=True, stop=S["bias_zero"][li],
                                         tile_position=(wc, 0))
                        if not S["bias_zero"][li]:
                            nc.tensor.matmul(out=mp[:], lhsT=ones_row[:, 0:128],
                                             rhs=ptile[f"bias_{li}"][:],
                                             start=False, stop=True)
                        cm = sbuf.tile([128, D], F32, tag="comb")
                        nc.vector.scalar_tensor_tensor(
                            out=cm[:], in0=at[:], scalar=invc_t[:, w:w + 1],
                            in1=mp[:], op0=ALU.mult, op1=ALU.add)
                        xn = sbuf.tile([128, D], F32, tag="xnew")
                        nc.scalar.activation(out=xn[:], in_=cm[:], func=AF.Relu)
                        transpose_to_own(xn, nxt_xT, w)
                        if w * 128 < OWN:
                            hi = min((w + 1) * 128, OWN)
                            wr = nc.sync.dma_start(
                                out=xbounce[li + 1][w * 128:hi, :],
                                in_=xn[0:hi - w * 128, :])
                            xw_written.append(wr)

                    ag = nc.gpsimd.collective_compute(
                        "AllGather", ALU.bypass, replica_groups=rg,
                        ins=[xbounce[li + 1].opt()], outs=[xfull[li + 1].opt()])
                    for wr in xw_written:
                        add_dep_helper(ag.ins, wr.ins, True, "ag after bounce")
                    prev_ag = ag

            # ---------- readout ----------
            with tc.tile_pool(name="ro", bufs=1) as ro:
                slabA = ro.tile([128, VAR_PAD], F32, tag="slabA")
                slabB = ro.tile([D, VAR_PAD], F32, tag="slabB")
                for l in range(5):
                    for t in range(NVT):
                        rt = sbuf.tile([128, D], F32, tag="ro_row")
                        gi = nc.gpsimd.indirect_dma_start(
                            out=rt[:], out_offset=None, in_=xfull[l][:],
                            in_offset=bass.IndirectOffsetOnAxis(
                                ap=vrow_t[:, t:t + 1], axis=0))
                        add_dep_helper(gi.ins, prev_ag.ins, True, "ro after ag")
                        tp = pt([D, 128])
                        nc.tensor.transpose(out=tp[:], in_=rt[:], identity=ident[:])
                        if l < 4:
                            dst = slabA[32 * l:32 * l + D, t * 128:(t + 1) * 128]
                        else:
                            dst = slabB[:, t * 128:(t + 1) * 128]
                        nc.scalar.activation(out=dst, in_=tp[:], func=AF.Copy)

                ls_slab = ro.tile([128, NVT * 2], F32, tag="lss")
                for ch in range(NRC):
                    cs = slice(ch * 512, (ch + 1) * 512)
                    hp = pa([D, 512])
                    nc.tensor.matmul(out=hp[:], lhsT=ptile["fc1a"][:],
                                     rhs=slabA[:, cs], start=True, stop=False)
                    nc.tensor.matmul(out=hp[:], lhsT=ptile["fc1b"][:],
                                     rhs=slabB[:, cs], start=False, stop=False)
                    nc.tensor.matmul(out=hp[:], lhsT=ptile["fc1bias"][:],
                                     rhs=ones_row[:], start=False, stop=True)
                    h = sbuf.tile([D, 512], F32, tag="fc_h")
                    nc.scalar.activation(out=h[:], in_=hp[:], func=AF.Relu)
                    for fck in ("fc2", "fc3"):
                        p2 = pa([D, 512])
                        nc.tensor.matmul(out=p2[:], lhsT=ptile[f"{fck}w"][:],
                                         rhs=h[:], start=True, stop=False)
                        nc.tensor.matmul(out=p2[:], lhsT=ptile[f"{fck}bias"][:],
                                         rhs=ones_row[:], start=False, stop=True)
                        h = sbuf.tile([D, 512], F32, tag="fc_h")
                        nc.scalar.activation(out=h[:], in_=p2[:], func=AF.Relu)
                    lp = pa([2, 512])
                    nc.tensor.matmul(out=lp[:], lhsT=ptile["fc6w"][:], rhs=h[:],
                                     start=True, stop=False)
                    nc.tensor.matmul(out=lp[:], lhsT=ptile["fc6bias"][:],
                                     rhs=ones_row[:], start=False, stop=True)
                    lsb = sbuf.tile([2, 512], F32, tag="fc6s")
                    nc.scalar.activation(out=lsb[:], in_=lp[:], func=AF.Copy)
                    for wi in range(4):
                        t = ch * 4 + wi
                        tp = pt([128, 2])
                        nc.tensor.transpose(out=tp[:],
                                            in_=lsb[:, wi * 128:(wi + 1) * 128],
                                            identity=ident[0:2, 0:2])
                        nc.scalar.activation(out=ls_slab[:, t * 2:(t + 1) * 2],
                                             in_=tp[:], func=AF.Copy)

                lsv = ls_slab[:].rearrange("p (t c) -> p t c", c=2)
                l0 = lsv[:, :, 0:1].rearrange("p t c -> p (t c)")
                l1 = lsv[:, :, 1:2].rearrange("p t c -> p (t c)")
                mx = sbuf.tile([128, NVT], F32, tag="ls_m")
                nc.vector.tensor_tensor(out=mx[:], in0=l0, in1=l1, op=ALU.max)
                zc = sbuf.tile([128, NVT * 2], F32, tag="ls_z")
                zcv = zc[:].rearrange("p (t c) -> p t c", c=2)
                mb = mx[:][:, :, None].to_broadcast([128, NVT, 2])
                nc.vector.tensor_tensor(out=zcv, in0=lsv, in1=mb, op=ALU.subtract)
                ec = sbuf.tile([128, NVT * 2], F32, tag="ls_e")
                nc.scalar.activation(out=ec[:], in_=zc[:], func=AF.Exp)
                ecv = ec[:].rearrange("p (t c) -> p t c", c=2)
                se = sbuf.tile([128, NVT], F32, tag="ls_s")
                nc.vector.tensor_tensor(
                    out=se[:],
                    in0=ecv[:, :, 0:1].rearrange("p t c -> p (t c)"),
                    in1=ecv[:, :, 1:2].rearrange("p t c -> p (t c)"),
                    op=ALU.add)
                lg = sbuf.tile([128, NVT], F32, tag="ls_l")
                nc.scalar.activation(out=lg[:], in_=se[:], func=AF.Ln)
                outt = sbuf.tile([128, NVT * 2], F32, tag="ls_o")
                ov_ = outt[:].rearrange("p (t c) -> p t c", c=2)
                lb = lg[:][:, :, None].to_broadcast([128, NVT, 2])
                nc.vector.tensor_tensor(out=ov_, in0=zcv, in1=lb, op=ALU.subtract)
                nc.sync.dma_start(
                    out=out_d.tensor.ap().rearrange("(t p) c -> p t c", p=128),
                    in_=ov_)

    nc.compile()
    return nc


# ---------------------------------------------------------------------------
_CACHE = {}


def kernel(**inputs):
    per_core, P, struct = _host_prep(**inputs)
    key = (struct["E_PAD"], struct["OWN_PAD"], struct["VAR_PAD"],
           tuple(struct["layer_ranges"]), tuple(struct["b2_zero"]),
           tuple(struct["bias_zero"]))
    if key not in _CACHE:
        _CACHE[key] = _build(struct, P)
    nc = _CACHE[key]

    NV, VARS, VAR_PAD = struct["NV"], struct["VARS"], struct["VAR_PAD"]
    NVT = VAR_PAD // 128
    in_maps = []
    for c in range(NCORES):
        m = dict(per_core[c])
        m.update(P)
        vr = np.arange(c * VARS, c * VARS + VAR_PAD) % NV
        m["vrowidx"] = np.ascontiguousarray(
            vr.reshape(NVT, 128).T.astype(np.int32))
        in_maps.append(m)

    res = run_bass_kernel_spmd(nc, in_maps, core_ids=list(range(NCORES)))
    out = np.concatenate(
        [res.results[c]["logits"][:VARS] for c in range(NCORES)], axis=0)
    kernel.last_results = res
    return np.ascontiguousarray(out.astype(np.float32))
